# revision 8
# baseline (speedup 1.0000x reference)
import numpy as np
import jax
import jax.numpy as jnp
from jax.sharding import Mesh, NamedSharding, PartitionSpec as P

try:
    from jax.experimental.shard_map import shard_map
except ImportError:
    from jax import shard_map

# nn_GTN_58205396795517: 2-layer TransformerConv GNN.
# Layout: dst-sharded. Core c owns nodes [c*NPAD, (c+1)*NPAD); edges are
# bucketed to the core owning their dst, so segment-softmax and the
# scatter-add are core-local (no all-reduce of node-indexed partials).
# Between layers a single all-gather replicates h [NTOT, D].
# Attention logits use q.k = x[dst] @ (Wq_h Wk_h^T) @ x[src]^T, so only the
# 64-wide x rows are gathered per edge (never 256-wide q/k rows), and the
# softmax normalization is deferred: agg = seg(ex*v)/seg(ex).
N = 100000
E = 800000
D = 64
H = 4
C = 64
M = 8
NPAD = 12544            # per-core node count (98*128)
NTOT = NPAD * M         # 100352

_INV_SQRT_C = np.float32(1.0 / np.sqrt(C))

_state = None


def _build():
    """Small single-pattern jits: the neuronx-cc backend crashes
    (DataLocalityOpt assert) on fused gather+arith graphs, so gathers are
    isolated; everything else is arith/matmul/segment_sum-only."""
    global _state
    if _state is not None:
        return _state
    mesh = Mesh(np.array(jax.devices()[:M]), ('x',))
    rep = NamedSharding(mesh, P())
    esh = NamedSharding(mesh, P('x'))

    def smap(fn, in_specs, out_specs, **kw):
        return jax.jit(shard_map(fn, mesh=mesh, in_specs=in_specs,
                                 out_specs=out_specs, **kw))

    J = {}
    # row gather from a replicated table
    J['gat_rep'] = smap(lambda t, i: t[i], (P(), P('x')), P('x'))
    # row gather from a row-sharded table with core-local indices
    J['gat_own'] = smap(lambda t, i: t[i], (P('x'), P('x')), P('x'))
    # per-edge projections: [Ccat | Wv] in one matmul
    J['yv'] = smap(lambda xs, CW: xs @ CW, (P('x'), P()), P('x'))
    # logits + exp (deferred-normalization softmax, exact bias terms)
    J['aexp'] = smap(
        lambda yv, xd, xs, U, W2, c0: jnp.exp(
            ((yv[:, :H * D].reshape(-1, H, D) * xd[:, None, :]).sum(-1)
             + xd @ U + xs @ W2 + c0) * _INV_SQRT_C),
        (P('x'), P('x'), P('x'), P(), P(), P()), P('x'))
    # weighted messages
    J['msg'] = smap(lambda yv, ex, bv:
                    (yv[:, H * D:] + bv) * jnp.repeat(ex, C, axis=1),
                    (P('x'), P('x'), P()), P('x'))
    # core-local segment sums (last segment = padding dump)
    J['seg'] = smap(lambda m, ex, i:
                    (jax.ops.segment_sum(m, i, num_segments=NPAD + 1)[:NPAD],
                     jax.ops.segment_sum(ex, i, num_segments=NPAD + 1)[:NPAD]),
                    (P('x'), P('x'), P('x')), (P('x'), P('x')))
    # normalize, head-mean, skip connection (+ optional relu)
    def fin(agg, den, xo, Ws, bs, do_relu):
        out = (agg.reshape(NPAD, H, C)
               / (den + 1e-16)[:, :, None]).mean(axis=1) + xo @ Ws + bs
        return jax.nn.relu(out) if do_relu else out
    J['fin_r'] = smap(lambda a, d, xo, Ws, bs: fin(a, d, xo, Ws, bs, True),
                      (P('x'),) * 3 + (P(), P()), P('x'))
    J['fin'] = smap(lambda a, d, xo, Ws, bs: fin(a, d, xo, Ws, bs, False),
                    (P('x'),) * 3 + (P(), P()), P('x'))
    # replicate shards between layers: place own block via one-hot, psum
    # (all_gather fails at runtime under this PJRT backend; psum is proven)
    J['ag'] = smap(lambda h, b: jax.lax.psum(
        (b.reshape(M, 1, 1) * h[None, :, :]).reshape(NTOT, D), 'x'),
        (P('x'), P('x')), P())

    _state = (mesh, rep, esh, J)
    return _state


def _prep_weights(Wq, bq, Wk, bk, Wv, bv):
    """Host-side exact refactor of per-head q.k dot into x-space."""
    Wq = np.asarray(Wq, np.float32); Wk = np.asarray(Wk, np.float32)
    Wv = np.asarray(Wv, np.float32)
    bq = np.asarray(bq, np.float32); bk = np.asarray(bk, np.float32)
    Cs, Us, W2s, c0s = [], [], [], []
    for h in range(H):
        Wq_h = Wq[:, h * C:(h + 1) * C]
        Wk_h = Wk[:, h * C:(h + 1) * C]
        bq_h = bq[h * C:(h + 1) * C]
        bk_h = bk[h * C:(h + 1) * C]
        Cs.append(Wk_h @ Wq_h.T)            # y_h = xs @ C_h ; alpha = y_h . xd
        Us.append(Wq_h @ bk_h)              # xd-linear bias term
        W2s.append(Wk_h @ bq_h)             # xs-linear bias term
        c0s.append(bq_h @ bk_h)
    CW = np.concatenate(Cs + [Wv], axis=1)              # [64, H*D + H*C]
    U = np.stack(Us, axis=1)                            # [64, H]
    W2 = np.stack(W2s, axis=1)                          # [64, H]
    c0 = np.array(c0s, np.float32)[None, :]             # [1, H]
    return CW, U, W2, c0, np.asarray(bv, np.float32)


def _prep_edges(edge_index):
    """Sort edges by dst, bucket by owning core, pad to a common length."""
    ei = np.asarray(edge_index)
    src, dst = ei[0].astype(np.int64), ei[1].astype(np.int64)
    order = np.argsort(dst, kind='stable')
    src, dst = src[order], dst[order]
    core = dst // NPAD
    counts = np.bincount(core, minlength=M)
    epad = int(-(-counts.max() // 128) * 128)
    src_a = np.zeros((M, epad), np.int32)
    locg_a = np.zeros((M, epad), np.int32)       # gather idx: pad -> 0 (in-bounds)
    locs_a = np.full((M, epad), NPAD, np.int32)  # segment idx: pad -> dump row
    start = 0
    for c in range(M):
        n = int(counts[c])
        src_a[c, :n] = src[start:start + n]
        locg_a[c, :n] = dst[start:start + n] - c * NPAD
        locs_a[c, :n] = locg_a[c, :n]
        start += n
    return src_a.reshape(-1), locg_a.reshape(-1), locs_a.reshape(-1)


def _layer(x_rep, x_own, srcs, locg, locs, CW, U, W2, c0, bv, Ws, bs, J, relu):
    xs = J['gat_rep'](x_rep, srcs)
    xd = J['gat_own'](x_own, locg)
    yv = J['yv'](xs, CW)
    ex = J['aexp'](yv, xd, xs, U, W2, c0)
    m = J['msg'](yv, ex, bv)
    agg, den = J['seg'](m, ex, locs)
    f = J['fin_r'] if relu else J['fin']
    return f(agg, den, x_own, Ws, bs)


def kernel(x, edge_index, Wq1, bq1, Wk1, bk1, Wv1, bv1, Ws1, bs1,
           Wq2, bq2, Wk2, bk2, Wv2, bv2, Ws2, bs2):
    mesh, rep, esh, J = _build()

    src_a, locg_a, locs_a = _prep_edges(edge_index)
    srcs = jax.device_put(jnp.asarray(src_a), esh)
    locg = jax.device_put(jnp.asarray(locg_a), esh)
    locs = jax.device_put(jnp.asarray(locs_a), esh)

    xp = np.zeros((NTOT, D), np.float32)
    xp[:N] = np.asarray(x, np.float32)
    x_rep = jax.device_put(jnp.asarray(xp), rep)
    x_own = jax.device_put(jnp.asarray(xp), esh)

    pr = lambda a: jax.device_put(jnp.asarray(np.asarray(a, np.float32)), rep)
    CW1, U1, W21, c01, bv1d = map(pr, _prep_weights(Wq1, bq1, Wk1, bk1, Wv1, bv1))
    CW2, U2, W22, c02, bv2d = map(pr, _prep_weights(Wq2, bq2, Wk2, bk2, Wv2, bv2))
    Ws1d, bs1d, Ws2d, bs2d = pr(Ws1), pr(bs1), pr(Ws2), pr(bs2)

    basis = jax.device_put(jnp.asarray(np.eye(M, dtype=np.float32)), esh)
    h_own = _layer(x_rep, x_own, srcs, locg, locs, CW1, U1, W21, c01, bv1d,
                   Ws1d, bs1d, J, relu=True)
    h_rep = J['ag'](h_own, basis)
    out_own = _layer(h_rep, h_own, srcs, locg, locs, CW2, U2, W22, c02, bv2d,
                     Ws2d, bs2d, J, relu=False)
    out = np.asarray(jax.device_get(out_own))
    return out[:N].astype(np.float32)


# revision 10
# speedup vs baseline: 1.5447x; 1.5447x over previous
import numpy as np
import jax
import jax.numpy as jnp
from jax.sharding import Mesh, NamedSharding, PartitionSpec as P

try:
    from jax.experimental.shard_map import shard_map
except ImportError:
    from jax import shard_map

# nn_GTN_58205396795517: 2-layer TransformerConv GNN.
# Layout: dst-sharded. Core c owns nodes [c*NPAD, (c+1)*NPAD); edges are
# bucketed to the core owning their dst, so segment-softmax and the
# scatter-add are core-local (no all-reduce of node-indexed partials).
# Between layers a single all-gather replicates h [NTOT, D].
# Attention logits use q.k = x[dst] @ (Wq_h Wk_h^T) @ x[src]^T, so only the
# 64-wide x rows are gathered per edge (never 256-wide q/k rows), and the
# softmax normalization is deferred: agg = seg(ex*v)/seg(ex).
N = 100000
E = 800000
D = 64
H = 4
C = 64
M = 8
NPAD = 12544            # per-core node count (98*128)
NTOT = NPAD * M         # 100352

_INV_SQRT_C = np.float32(1.0 / np.sqrt(C))

_state = None


def _build():
    """Small single-pattern jits: the neuronx-cc backend crashes
    (DataLocalityOpt assert) on fused gather+arith graphs, so gathers are
    isolated; everything else is arith/matmul/segment_sum-only."""
    global _state
    if _state is not None:
        return _state
    mesh = Mesh(np.array(jax.devices()[:M]), ('x',))
    rep = NamedSharding(mesh, P())
    esh = NamedSharding(mesh, P('x'))

    def smap(fn, in_specs, out_specs, **kw):
        return jax.jit(shard_map(fn, mesh=mesh, in_specs=in_specs,
                                 out_specs=out_specs, **kw))

    J = {}
    # row gather from a replicated table
    J['gat_rep'] = smap(lambda t, i: t[i], (P(), P('x')), P('x'))
    # row gather from a row-sharded table with core-local indices
    J['gat_own'] = smap(lambda t, i: t[i], (P('x'), P('x')), P('x'))
    # per-edge projections: [Ccat | Wv] in one matmul
    J['yv'] = smap(lambda xs, CW: xs @ CW, (P('x'), P()), P('x'))
    # logits + exp (deferred-normalization softmax, exact bias terms)
    J['aexp'] = smap(
        lambda yv, xd, xs, U, W2, c0: jnp.exp(
            ((yv[:, :H * D].reshape(-1, H, D) * xd[:, None, :]).sum(-1)
             + xd @ U + xs @ W2 + c0) * _INV_SQRT_C),
        (P('x'), P('x'), P('x'), P(), P(), P()), P('x'))
    # weighted messages
    J['msg'] = smap(lambda yv, ex, bv:
                    (yv[:, H * D:] + bv) * jnp.repeat(ex, C, axis=1),
                    (P('x'), P('x'), P()), P('x'))
    # core-local segment sums (last segment = padding dump)
    J['seg'] = smap(lambda m, ex, i:
                    (jax.ops.segment_sum(m, i, num_segments=NPAD + 1)[:NPAD],
                     jax.ops.segment_sum(ex, i, num_segments=NPAD + 1)[:NPAD]),
                    (P('x'), P('x'), P('x')), (P('x'), P('x')))
    # normalize, head-mean, skip connection (+ optional relu)
    def fin(agg, den, xo, Ws, bs, do_relu):
        out = (agg.reshape(NPAD, H, C)
               / (den + 1e-16)[:, :, None]).mean(axis=1) + xo @ Ws + bs
        return jax.nn.relu(out) if do_relu else out
    J['fin_r'] = smap(lambda a, d, xo, Ws, bs: fin(a, d, xo, Ws, bs, True),
                      (P('x'),) * 3 + (P(), P()), P('x'))
    J['fin'] = smap(lambda a, d, xo, Ws, bs: fin(a, d, xo, Ws, bs, False),
                    (P('x'),) * 3 + (P(), P()), P('x'))
    # replicate shards between layers: place own block via one-hot, psum
    # (all_gather fails at runtime under this PJRT backend; psum is proven)
    J['ag'] = smap(lambda h, b: jax.lax.psum(
        (b.reshape(M, 1, 1) * h[None, :, :]).reshape(NTOT, D), 'x'),
        (P('x'), P('x')), P())

    # fully fused 2-layer pipeline: one dispatch (launch overhead dominates)
    def _lay(x_rep_, x_own_, srcs, locg, locs, CW, U, W2, c0, bv, Ws, bs,
             do_relu):
        xs = x_rep_[srcs]
        xd = x_own_[locg]
        yv = xs @ CW
        ex = jnp.exp(((yv[:, :H * D].reshape(-1, H, D)
                       * xd[:, None, :]).sum(-1)
                      + xd @ U + xs @ W2 + c0) * _INV_SQRT_C)
        m = (yv[:, H * D:] + bv) * jnp.repeat(ex, C, axis=1)
        agg = jax.ops.segment_sum(m, locs, num_segments=NPAD + 1)[:NPAD]
        den = jax.ops.segment_sum(ex, locs, num_segments=NPAD + 1)[:NPAD]
        out = (agg.reshape(NPAD, H, C)
               / (den + 1e-16)[:, :, None]).mean(axis=1) + x_own_ @ Ws + bs
        return jax.nn.relu(out) if do_relu else out

    def _full(x_rep_, x_own_, srcs, locg, locs, basis, w1, w2):
        h = _lay(x_rep_, x_own_, srcs, locg, locs, *w1, True)
        h_rep = jax.lax.psum(
            (basis.reshape(M, 1, 1) * h[None, :, :]).reshape(NTOT, D), 'x')
        return _lay(h_rep, h, srcs, locg, locs, *w2, False)

    J['full'] = smap(_full,
                     (P(), P('x'), P('x'), P('x'), P('x'), P('x'),
                      (P(),) * 7, (P(),) * 7), P('x'))

    _state = (mesh, rep, esh, J)
    return _state


def _prep_weights(Wq, bq, Wk, bk, Wv, bv):
    """Host-side exact refactor of per-head q.k dot into x-space."""
    Wq = np.asarray(Wq, np.float32); Wk = np.asarray(Wk, np.float32)
    Wv = np.asarray(Wv, np.float32)
    bq = np.asarray(bq, np.float32); bk = np.asarray(bk, np.float32)
    Cs, Us, W2s, c0s = [], [], [], []
    for h in range(H):
        Wq_h = Wq[:, h * C:(h + 1) * C]
        Wk_h = Wk[:, h * C:(h + 1) * C]
        bq_h = bq[h * C:(h + 1) * C]
        bk_h = bk[h * C:(h + 1) * C]
        Cs.append(Wk_h @ Wq_h.T)            # y_h = xs @ C_h ; alpha = y_h . xd
        Us.append(Wq_h @ bk_h)              # xd-linear bias term
        W2s.append(Wk_h @ bq_h)             # xs-linear bias term
        c0s.append(bq_h @ bk_h)
    CW = np.concatenate(Cs + [Wv], axis=1)              # [64, H*D + H*C]
    U = np.stack(Us, axis=1)                            # [64, H]
    W2 = np.stack(W2s, axis=1)                          # [64, H]
    c0 = np.array(c0s, np.float32)[None, :]             # [1, H]
    return CW, U, W2, c0, np.asarray(bv, np.float32)


def _prep_edges(edge_index):
    """Sort edges by dst, bucket by owning core, pad to a common length."""
    ei = np.asarray(edge_index)
    src, dst = ei[0].astype(np.int64), ei[1].astype(np.int64)
    order = np.argsort(dst, kind='stable')
    src, dst = src[order], dst[order]
    core = dst // NPAD
    counts = np.bincount(core, minlength=M)
    epad = int(-(-counts.max() // 128) * 128)
    src_a = np.zeros((M, epad), np.int32)
    locg_a = np.zeros((M, epad), np.int32)       # gather idx: pad -> 0 (in-bounds)
    locs_a = np.full((M, epad), NPAD, np.int32)  # segment idx: pad -> dump row
    start = 0
    for c in range(M):
        n = int(counts[c])
        src_a[c, :n] = src[start:start + n]
        locg_a[c, :n] = dst[start:start + n] - c * NPAD
        locs_a[c, :n] = locg_a[c, :n]
        start += n
    return src_a.reshape(-1), locg_a.reshape(-1), locs_a.reshape(-1)


def _layer(x_rep, x_own, srcs, locg, locs, CW, U, W2, c0, bv, Ws, bs, J, relu):
    xs = J['gat_rep'](x_rep, srcs)
    xd = J['gat_own'](x_own, locg)
    yv = J['yv'](xs, CW)
    ex = J['aexp'](yv, xd, xs, U, W2, c0)
    m = J['msg'](yv, ex, bv)
    agg, den = J['seg'](m, ex, locs)
    f = J['fin_r'] if relu else J['fin']
    return f(agg, den, x_own, Ws, bs)


def kernel(x, edge_index, Wq1, bq1, Wk1, bk1, Wv1, bv1, Ws1, bs1,
           Wq2, bq2, Wk2, bk2, Wv2, bv2, Ws2, bs2):
    mesh, rep, esh, J = _build()

    src_a, locg_a, locs_a = _prep_edges(edge_index)
    srcs = jax.device_put(jnp.asarray(src_a), esh)
    locg = jax.device_put(jnp.asarray(locg_a), esh)
    locs = jax.device_put(jnp.asarray(locs_a), esh)

    xp = np.zeros((NTOT, D), np.float32)
    xp[:N] = np.asarray(x, np.float32)
    x_rep = jax.device_put(jnp.asarray(xp), rep)
    x_own = jax.device_put(jnp.asarray(xp), esh)

    pr = lambda a: jax.device_put(jnp.asarray(np.asarray(a, np.float32)), rep)
    CW1, U1, W21, c01, bv1d = map(pr, _prep_weights(Wq1, bq1, Wk1, bk1, Wv1, bv1))
    CW2, U2, W22, c02, bv2d = map(pr, _prep_weights(Wq2, bq2, Wk2, bk2, Wv2, bv2))
    Ws1d, bs1d, Ws2d, bs2d = pr(Ws1), pr(bs1), pr(Ws2), pr(bs2)

    basis = jax.device_put(jnp.asarray(np.eye(M, dtype=np.float32)), esh)
    w1 = (CW1, U1, W21, c01, bv1d, Ws1d, bs1d)
    w2 = (CW2, U2, W22, c02, bv2d, Ws2d, bs2d)
    try:
        out_own = J['full'](x_rep, x_own, srcs, locg, locs, basis, w1, w2)
        out_own.block_until_ready()
    except Exception:
        # fused graph unsupported by the backend: stage-by-stage fallback
        h_own = _layer(x_rep, x_own, srcs, locg, locs, CW1, U1, W21, c01,
                       bv1d, Ws1d, bs1d, J, relu=True)
        h_rep = J['ag'](h_own, basis)
        out_own = _layer(h_rep, h_own, srcs, locg, locs, CW2, U2, W22, c02,
                         bv2d, Ws2d, bs2d, J, relu=False)
    out = np.asarray(jax.device_get(out_own))
    return out[:N].astype(np.float32)


# revision 11
# speedup vs baseline: 1.6124x; 1.0438x over previous
import numpy as np
import jax
import jax.numpy as jnp
from jax.sharding import Mesh, NamedSharding, PartitionSpec as P

try:
    from jax.experimental.shard_map import shard_map
except ImportError:
    from jax import shard_map

# nn_GTN_58205396795517: 2-layer TransformerConv GNN.
#
# Layout: dst-sharded. Core c owns nodes [c*NPAD, (c+1)*NPAD); edges are
# bucketed to the core owning their dst, so segment-softmax and scatter-add
# are core-local. Between layers one psum replicates h [NTOT, D].
# Logits use q.k = x[dst] @ (Wq_h Wk_h^T) @ x[src]^T so only 64-wide x rows
# are gathered per edge (never 256-wide q/k rows); softmax normalization is
# deferred: out = seg(ex*v)/seg(ex).
#
# Dispatch count dominates wall time on this backend (~20-40ms/launch), so
# each layer is exactly two dispatches: one combined [src;dst] row gather
# (gathers fused with arith produce NEFFs that wedge the device, so the
# gather stays isolated) and one fused everything-else stage.
N = 100000
E = 800000
D = 64
H = 4
C = 64
M = 8
NPAD = 12544            # per-core node count (98*128)
NTOT = NPAD * M         # 100352

_INV_SQRT_C = np.float32(1.0 / np.sqrt(C))

_state = None


def _build():
    global _state
    if _state is not None:
        return _state
    mesh = Mesh(np.array(jax.devices()[:M]), ('x',))
    rep = NamedSharding(mesh, P())
    esh = NamedSharding(mesh, P('x'))

    def smap(fn, in_specs, out_specs):
        return jax.jit(shard_map(fn, mesh=mesh, in_specs=in_specs,
                                 out_specs=out_specs))

    J = {}
    # one combined row gather from the replicated node table
    J['gat'] = smap(lambda t, i: t[i], (P(), P('x')), P('x'))

    # everything else for one layer, fused: projections, logits+exp,
    # messages, local segment sums, normalize+skip (+relu, +psum-replicate)
    def _rest(g, x_own_, locs, basis, CW, U, W2, c0, bv, Ws, bs,
              do_relu, do_ag):
        ep = g.shape[0] // 2
        xs, xd = g[:ep], g[ep:]
        yv = xs @ CW
        ex = jnp.exp(((yv[:, :H * D].reshape(-1, H, D)
                       * xd[:, None, :]).sum(-1)
                      + xd @ U + xs @ W2 + c0) * _INV_SQRT_C)
        m = (yv[:, H * D:] + bv) * jnp.repeat(ex, C, axis=1)
        agg = jax.ops.segment_sum(m, locs, num_segments=NPAD + 1)[:NPAD]
        den = jax.ops.segment_sum(ex, locs, num_segments=NPAD + 1)[:NPAD]
        out = (agg.reshape(NPAD, H, C)
               / (den + 1e-16)[:, :, None]).mean(axis=1) + x_own_ @ Ws + bs
        if do_relu:
            out = jax.nn.relu(out)
        if do_ag:
            out_rep = jax.lax.psum(
                (basis.reshape(M, 1, 1) * out[None, :, :]).reshape(NTOT, D),
                'x')
            return out, out_rep
        return out

    rest_specs = (P('x'), P('x'), P('x'), P('x')) + (P(),) * 7
    J['rest1'] = smap(lambda *a: _rest(*a, True, True),
                      rest_specs, (P('x'), P()))
    J['rest2'] = smap(lambda *a: _rest(*a, False, False),
                      rest_specs, P('x'))

    _state = (mesh, rep, esh, J)
    return _state


def _prep_weights(Wq, bq, Wk, bk, Wv, bv):
    """Host-side exact refactor of the per-head q.k dot into x-space."""
    Wq = np.asarray(Wq, np.float32); Wk = np.asarray(Wk, np.float32)
    Wv = np.asarray(Wv, np.float32)
    bq = np.asarray(bq, np.float32); bk = np.asarray(bk, np.float32)
    Cs, Us, W2s, c0s = [], [], [], []
    for h in range(H):
        Wq_h = Wq[:, h * C:(h + 1) * C]
        Wk_h = Wk[:, h * C:(h + 1) * C]
        bq_h = bq[h * C:(h + 1) * C]
        bk_h = bk[h * C:(h + 1) * C]
        Cs.append(Wk_h @ Wq_h.T)        # y_h = xs @ C_h ; alpha_h = y_h . xd
        Us.append(Wq_h @ bk_h)          # xd-linear bias term
        W2s.append(Wk_h @ bq_h)         # xs-linear bias term
        c0s.append(bq_h @ bk_h)
    CW = np.concatenate(Cs + [Wv], axis=1)              # [D, H*D + H*C]
    U = np.stack(Us, axis=1)                            # [D, H]
    W2 = np.stack(W2s, axis=1)                          # [D, H]
    c0 = np.array(c0s, np.float32)[None, :]             # [1, H]
    return CW, U, W2, c0, np.asarray(bv, np.float32)


def _prep_edges(edge_index):
    """Sort edges by dst, bucket by owning core, pad to a common length.
    Returns the combined [src; dst_global] gather index array and the
    segment index array (pad edges dump into segment NPAD)."""
    ei = np.asarray(edge_index)
    src, dst = ei[0].astype(np.int64), ei[1].astype(np.int64)
    order = np.argsort(dst, kind='stable')
    src, dst = src[order], dst[order]
    core = dst // NPAD
    counts = np.bincount(core, minlength=M)
    epad = int(-(-counts.max() // 128) * 128)
    gidx = np.zeros((M, 2 * epad), np.int32)     # [src rows | dst rows]
    locs = np.full((M, epad), NPAD, np.int32)    # segment idx: pad -> dump
    start = 0
    for c in range(M):
        n = int(counts[c])
        gidx[c, :n] = src[start:start + n]
        gidx[c, epad:epad + n] = dst[start:start + n]
        gidx[c, epad + n:] = c * NPAD            # in-bounds pad (own row 0)
        locs[c, :n] = dst[start:start + n] - c * NPAD
        start += n
    return gidx.reshape(-1), locs.reshape(-1), epad


def kernel(x, edge_index, Wq1, bq1, Wk1, bk1, Wv1, bv1, Ws1, bs1,
           Wq2, bq2, Wk2, bk2, Wv2, bv2, Ws2, bs2):
    mesh, rep, esh, J = _build()

    gidx_a, locs_a, _ = _prep_edges(edge_index)
    gidx = jax.device_put(jnp.asarray(gidx_a), esh)
    locs = jax.device_put(jnp.asarray(locs_a), esh)

    xp = np.zeros((NTOT, D), np.float32)
    xp[:N] = np.asarray(x, np.float32)
    x_rep = jax.device_put(jnp.asarray(xp), rep)
    x_own = jax.device_put(jnp.asarray(xp), esh)

    pr = lambda a: jax.device_put(jnp.asarray(np.asarray(a, np.float32)), rep)
    w1 = tuple(map(pr, _prep_weights(Wq1, bq1, Wk1, bk1, Wv1, bv1))) \
        + (pr(Ws1), pr(bs1))
    w2 = tuple(map(pr, _prep_weights(Wq2, bq2, Wk2, bk2, Wv2, bv2))) \
        + (pr(Ws2), pr(bs2))
    basis = jax.device_put(jnp.asarray(np.eye(M, dtype=np.float32)), esh)

    g1 = J['gat'](x_rep, gidx)
    h_own, h_rep = J['rest1'](g1, x_own, locs, basis, *w1)
    g2 = J['gat'](h_rep, gidx)
    out_own = J['rest2'](g2, h_own, locs, basis, *w2)
    out = np.asarray(jax.device_get(out_own))
    return out[:N].astype(np.float32)


# revision 12
# speedup vs baseline: 1.6311x; 1.0116x over previous
import numpy as np
import jax
import jax.numpy as jnp
from jax.sharding import Mesh, NamedSharding, PartitionSpec as P

try:
    from jax.experimental.shard_map import shard_map
except ImportError:
    from jax import shard_map

# nn_GTN_58205396795517: 2-layer TransformerConv GNN.
#
# Layout: dst-sharded. Core c owns nodes [c*NPAD, (c+1)*NPAD); edges are
# bucketed to the core owning their dst, so segment-softmax and scatter-add
# are core-local. Between layers one psum replicates h [NTOT, D].
# Logits use q.k = x[dst] @ (Wq_h Wk_h^T) @ x[src]^T so only 64-wide x rows
# are gathered per edge (never 256-wide q/k rows); softmax normalization is
# deferred: out = seg(ex*v)/seg(ex).
#
# Dispatch count dominates wall time on this backend (~20-40ms/launch), so
# each layer is exactly two dispatches: one combined [src;dst] row gather
# (gathers fused with arith produce NEFFs that wedge the device, so the
# gather stays isolated) and one fused everything-else stage.
N = 100000
E = 800000
D = 64
H = 4
C = 64
M = 8
NPAD = 12544            # per-core node count (98*128)
NTOT = NPAD * M         # 100352

_INV_SQRT_C = np.float32(1.0 / np.sqrt(C))

_state = None


def _build():
    global _state
    if _state is not None:
        return _state
    mesh = Mesh(np.array(jax.devices()[:M]), ('x',))
    rep = NamedSharding(mesh, P())
    esh = NamedSharding(mesh, P('x'))

    def smap(fn, in_specs, out_specs):
        return jax.jit(shard_map(fn, mesh=mesh, in_specs=in_specs,
                                 out_specs=out_specs))

    J = {}
    # one combined row gather from the replicated node table
    J['gat'] = smap(lambda t, i: t[i], (P(), P('x')), P('x'))

    # everything else for one layer, fused: projections, logits+exp,
    # messages, local segment sums, normalize+skip (+relu, +psum-replicate)
    def _rest(g, x_own_, locs, basis, CW, U, W2, c0, bv, Ws, bs,
              do_relu, do_ag):
        ep = g.shape[0] // 2
        xs = g[:ep].astype(jnp.float32)
        xd = g[ep:].astype(jnp.float32)
        yv = xs @ CW
        ex = jnp.exp(((yv[:, :H * D].reshape(-1, H, D)
                       * xd[:, None, :]).sum(-1)
                      + xd @ U + xs @ W2 + c0) * _INV_SQRT_C)
        m = (yv[:, H * D:] + bv) * jnp.repeat(ex, C, axis=1)
        agg = jax.ops.segment_sum(m, locs, num_segments=NPAD + 1)[:NPAD]
        den = jax.ops.segment_sum(ex, locs, num_segments=NPAD + 1)[:NPAD]
        out = (agg.reshape(NPAD, H, C)
               / (den + 1e-16)[:, :, None]).mean(axis=1) + x_own_ @ Ws + bs
        if do_relu:
            out = jax.nn.relu(out)
        if do_ag:
            out_rep = jax.lax.psum(
                (basis.reshape(M, 1, 1)
                 * out[None, :, :]).reshape(NTOT, D).astype(jnp.bfloat16),
                'x')
            return out, out_rep
        return out

    rest_specs = (P('x'), P('x'), P('x'), P('x')) + (P(),) * 7
    J['rest1'] = smap(lambda *a: _rest(*a, True, True),
                      rest_specs, (P('x'), P()))
    J['rest2'] = smap(lambda *a: _rest(*a, False, False),
                      rest_specs, P('x'))

    _state = (mesh, rep, esh, J)
    return _state


def _prep_weights(Wq, bq, Wk, bk, Wv, bv):
    """Host-side exact refactor of the per-head q.k dot into x-space."""
    Wq = np.asarray(Wq, np.float32); Wk = np.asarray(Wk, np.float32)
    Wv = np.asarray(Wv, np.float32)
    bq = np.asarray(bq, np.float32); bk = np.asarray(bk, np.float32)
    Cs, Us, W2s, c0s = [], [], [], []
    for h in range(H):
        Wq_h = Wq[:, h * C:(h + 1) * C]
        Wk_h = Wk[:, h * C:(h + 1) * C]
        bq_h = bq[h * C:(h + 1) * C]
        bk_h = bk[h * C:(h + 1) * C]
        Cs.append(Wk_h @ Wq_h.T)        # y_h = xs @ C_h ; alpha_h = y_h . xd
        Us.append(Wq_h @ bk_h)          # xd-linear bias term
        W2s.append(Wk_h @ bq_h)         # xs-linear bias term
        c0s.append(bq_h @ bk_h)
    CW = np.concatenate(Cs + [Wv], axis=1)              # [D, H*D + H*C]
    U = np.stack(Us, axis=1)                            # [D, H]
    W2 = np.stack(W2s, axis=1)                          # [D, H]
    c0 = np.array(c0s, np.float32)[None, :]             # [1, H]
    return CW, U, W2, c0, np.asarray(bv, np.float32)


def _prep_edges(edge_index):
    """Sort edges by dst, bucket by owning core, pad to a common length.
    Returns the combined [src; dst_global] gather index array and the
    segment index array (pad edges dump into segment NPAD)."""
    ei = np.asarray(edge_index)
    src, dst = ei[0].astype(np.int64), ei[1].astype(np.int64)
    order = np.argsort(dst, kind='stable')
    src, dst = src[order], dst[order]
    core = dst // NPAD
    counts = np.bincount(core, minlength=M)
    epad = int(-(-counts.max() // 128) * 128)
    gidx = np.zeros((M, 2 * epad), np.int32)     # [src rows | dst rows]
    locs = np.full((M, epad), NPAD, np.int32)    # segment idx: pad -> dump
    start = 0
    for c in range(M):
        n = int(counts[c])
        gidx[c, :n] = src[start:start + n]
        gidx[c, epad:epad + n] = dst[start:start + n]
        gidx[c, epad + n:] = c * NPAD            # in-bounds pad (own row 0)
        locs[c, :n] = dst[start:start + n] - c * NPAD
        start += n
    return gidx.reshape(-1), locs.reshape(-1), epad


def kernel(x, edge_index, Wq1, bq1, Wk1, bk1, Wv1, bv1, Ws1, bs1,
           Wq2, bq2, Wk2, bk2, Wv2, bv2, Ws2, bs2):
    mesh, rep, esh, J = _build()

    gidx_a, locs_a, _ = _prep_edges(edge_index)
    gidx = jax.device_put(jnp.asarray(gidx_a), esh)
    locs = jax.device_put(jnp.asarray(locs_a), esh)

    xp = np.zeros((NTOT, D), np.float32)
    xp[:N] = np.asarray(x, np.float32)
    x_rep = jax.device_put(jnp.asarray(xp, dtype=jnp.bfloat16), rep)
    x_own = jax.device_put(jnp.asarray(xp), esh)

    pr = lambda a: jax.device_put(jnp.asarray(np.asarray(a, np.float32)), rep)
    w1 = tuple(map(pr, _prep_weights(Wq1, bq1, Wk1, bk1, Wv1, bv1))) \
        + (pr(Ws1), pr(bs1))
    w2 = tuple(map(pr, _prep_weights(Wq2, bq2, Wk2, bk2, Wv2, bv2))) \
        + (pr(Ws2), pr(bs2))
    basis = jax.device_put(jnp.asarray(np.eye(M, dtype=np.float32)), esh)

    g1 = J['gat'](x_rep, gidx)
    h_own, h_rep = J['rest1'](g1, x_own, locs, basis, *w1)
    g2 = J['gat'](h_rep, gidx)
    out_own = J['rest2'](g2, h_own, locs, basis, *w2)
    out = np.asarray(jax.device_get(out_own))
    return out[:N].astype(np.float32)


# revision 14
# speedup vs baseline: 2.0176x; 1.2370x over previous
import numpy as np
import jax
import jax.numpy as jnp
from jax.sharding import Mesh, NamedSharding, PartitionSpec as P

try:
    from jax.experimental.shard_map import shard_map
except ImportError:
    from jax import shard_map

# nn_GTN_58205396795517: 2-layer TransformerConv GNN.
#
# Layout: dst-sharded. Core c owns nodes [c*NPAD, (c+1)*NPAD); edges are
# bucketed to the core owning their dst, so segment-softmax and scatter-add
# are core-local. Between layers one psum replicates h [NTOT, D].
# Logits use q.k = x[dst] @ (Wq_h Wk_h^T) @ x[src]^T so only 64-wide x rows
# are gathered per edge (never 256-wide q/k rows); softmax normalization is
# deferred: out = seg(ex*v)/seg(ex).
#
# Dispatch count dominates wall time on this backend (~20-40ms/launch), so
# each layer is exactly two dispatches: one combined [src;dst] row gather
# (gathers fused with arith produce NEFFs that wedge the device, so the
# gather stays isolated) and one fused everything-else stage.
N = 100000
E = 800000
D = 64
H = 4
C = 64
M = 8
NPAD = 12544            # per-core node count (98*128)
NTOT = NPAD * M         # 100352

_INV_SQRT_C = np.float32(1.0 / np.sqrt(C))

_state = None


def _build():
    global _state
    if _state is not None:
        return _state
    mesh = Mesh(np.array(jax.devices()[:M]), ('x',))
    rep = NamedSharding(mesh, P())
    esh = NamedSharding(mesh, P('x'))

    def smap(fn, in_specs, out_specs):
        return jax.jit(shard_map(fn, mesh=mesh, in_specs=in_specs,
                                 out_specs=out_specs))

    J = {}
    # one combined row gather from the replicated node table
    J['gat'] = smap(lambda t, i: t[i], (P(), P('x')), P('x'))

    # everything else for one layer, fused: projections, logits+exp,
    # messages, local segment sums, normalize+skip (+relu, +psum-replicate)
    def _rest(g, x_own_, locs, basis, CW, U, W2, c0, bv, Ws, bs,
              do_relu, do_ag):
        ep = g.shape[0] // 2
        xs, xd = g[:ep], g[ep:]
        yv = xs @ CW
        ex = jnp.exp(((yv[:, :H * D].reshape(-1, H, D)
                       * xd[:, None, :]).sum(-1)
                      + xd @ U + xs @ W2 + c0) * _INV_SQRT_C)
        m = (yv[:, H * D:] + bv) * jnp.repeat(ex, C, axis=1)
        agg = jax.ops.segment_sum(m, locs, num_segments=NPAD + 1)[:NPAD]
        den = jax.ops.segment_sum(ex, locs, num_segments=NPAD + 1)[:NPAD]
        out = (agg.reshape(NPAD, H, C)
               / (den + 1e-16)[:, :, None]).mean(axis=1) + x_own_ @ Ws + bs
        if do_relu:
            out = jax.nn.relu(out)
        if do_ag:
            out_rep = jax.lax.psum(
                (basis.reshape(M, 1, 1) * out[None, :, :]).reshape(NTOT, D),
                'x')
            return out, out_rep
        return out

    rest_specs = (P('x'), P('x'), P('x'), P('x')) + (P(),) * 7
    J['rest1'] = smap(lambda *a: _rest(*a, True, True),
                      rest_specs, (P('x'), P()))
    J['rest2'] = smap(lambda *a: _rest(*a, False, False),
                      rest_specs, P('x'))

    _state = (mesh, rep, esh, J)
    return _state


def _prep_weights(Wq, bq, Wk, bk, Wv, bv):
    """Host-side exact refactor of the per-head q.k dot into x-space."""
    Wq = np.asarray(Wq, np.float32); Wk = np.asarray(Wk, np.float32)
    Wv = np.asarray(Wv, np.float32)
    bq = np.asarray(bq, np.float32); bk = np.asarray(bk, np.float32)
    Cs, Us, W2s, c0s = [], [], [], []
    for h in range(H):
        Wq_h = Wq[:, h * C:(h + 1) * C]
        Wk_h = Wk[:, h * C:(h + 1) * C]
        bq_h = bq[h * C:(h + 1) * C]
        bk_h = bk[h * C:(h + 1) * C]
        Cs.append(Wk_h @ Wq_h.T)        # y_h = xs @ C_h ; alpha_h = y_h . xd
        Us.append(Wq_h @ bk_h)          # xd-linear bias term
        W2s.append(Wk_h @ bq_h)         # xs-linear bias term
        c0s.append(bq_h @ bk_h)
    CW = np.concatenate(Cs + [Wv], axis=1)              # [D, H*D + H*C]
    U = np.stack(Us, axis=1)                            # [D, H]
    W2 = np.stack(W2s, axis=1)                          # [D, H]
    c0 = np.array(c0s, np.float32)[None, :]             # [1, H]
    return CW, U, W2, c0, np.asarray(bv, np.float32)


def _prep_edges(edge_index):
    """Sort edges by dst, bucket by owning core, pad to a common length.
    Returns the combined [src; dst_global] gather index array and the
    segment index array (pad edges dump into segment NPAD)."""
    ei = np.asarray(edge_index)
    src, dst = ei[0].astype(np.int64), ei[1].astype(np.int64)
    order = np.argsort(dst, kind='stable')
    src, dst = src[order], dst[order]
    core = dst // NPAD
    counts = np.bincount(core, minlength=M)
    epad = int(-(-counts.max() // 128) * 128)
    gidx = np.zeros((M, 2 * epad), np.int32)     # [src rows | dst rows]
    locs = np.full((M, epad), NPAD, np.int32)    # segment idx: pad -> dump
    start = 0
    for c in range(M):
        n = int(counts[c])
        gidx[c, :n] = src[start:start + n]
        gidx[c, epad:epad + n] = dst[start:start + n]
        gidx[c, epad + n:] = c * NPAD            # in-bounds pad (own row 0)
        locs[c, :n] = dst[start:start + n] - c * NPAD
        start += n
    return gidx.reshape(-1), locs.reshape(-1), epad


def kernel(x, edge_index, Wq1, bq1, Wk1, bk1, Wv1, bv1, Ws1, bs1,
           Wq2, bq2, Wk2, bk2, Wv2, bv2, Ws2, bs2):
    mesh, rep, esh, J = _build()

    gidx_a, locs_a, _ = _prep_edges(edge_index)
    gidx = jax.device_put(jnp.asarray(gidx_a), esh)
    locs = jax.device_put(jnp.asarray(locs_a), esh)

    xp = np.zeros((NTOT, D), np.float32)
    xp[:N] = np.asarray(x, np.float32)
    x_own = jax.device_put(jnp.asarray(xp), esh)

    pr = lambda a: jax.device_put(jnp.asarray(np.asarray(a, np.float32)), rep)
    w1 = tuple(map(pr, _prep_weights(Wq1, bq1, Wk1, bk1, Wv1, bv1))) \
        + (pr(Ws1), pr(bs1))
    w2 = tuple(map(pr, _prep_weights(Wq2, bq2, Wk2, bk2, Wv2, bv2))) \
        + (pr(Ws2), pr(bs2))
    basis = jax.device_put(jnp.asarray(np.eye(M, dtype=np.float32)), esh)

    # layer-1 gather has static input: do it on the host (input prep),
    # saving one of the two expensive device gather dispatches
    g1_host = xp[gidx_a]                       # [M*2EP, D]
    g1 = jax.device_put(jnp.asarray(g1_host), esh)
    h_own, h_rep = J['rest1'](g1, x_own, locs, basis, *w1)
    g2 = J['gat'](h_rep, gidx)
    out_own = J['rest2'](g2, h_own, locs, basis, *w2)
    out = np.asarray(jax.device_get(out_own))
    return out[:N].astype(np.float32)
